# revision 8
# baseline (speedup 1.0000x reference)
"""Trainium2 Bass kernel for causal multi-head attention with RoPE.

Reference computation (B=2, S=2048, D=1024, H=16, DH=64, fp32):
    qkv = x @ w_qkv ; q,k,v = split(qkv)
    q,k = rope(q), rope(k)
    out = causal_sdpa(q, k, v, scale=DH**-0.5) @ w_out

Sharding over 8 NeuronCores: data-parallel on batch (2 groups of 4 cores),
tensor-parallel on heads (4 heads/core; QKV projection columns and out
projection rows sharded accordingly).  Each core emits a partial [S, D]
output; the host sums the 4 partials per batch (the TP all-reduce).

Device-side layout tricks:
  - host passes x TRANSPOSED ([D, S]) so all matmuls consume natural
    DRAM layouts without any on-device fp32 transposes
  - q, k are computed channel-major ("qT" [dh, s]); scores are computed
    transposed (key position on PSUM partitions), so softmax's sum folds
    into the AV matmul via ones-columns appended to V, and no transposes
    of the probability matrix are needed
  - RoPE's rotate_half is folded into the QKV projection by also
    projecting against half-rotated weight columns (host-prepared), so no
    cross-partition shuffles are needed
  - softmax skips the max-subtraction (scores are O(+-8); fp32 exp is
    exact enough); masked logits get -1e9 bias before exp
  - matmuls run in float32r (fp32 storage, reduced-precision PE path,
    4x the fp32 matmul throughput; measured ~1.5e-4 rel err at K=1024)

Self-contained: hardcodes all shapes; no sibling imports.
"""

import os
import sys

sys.path.insert(0, "/opt/trn_rl_repo")

import numpy as np
from contextlib import ExitStack

import concourse.bass as bass
import concourse.tile as tile
from concourse import bacc, mybir

P = 128
B = 2
S = 2048
D = 1024
H = 16          # total heads
NH = 4          # heads per core
DH = 64
KB = D // P     # 8 contraction blocks
SC = 512        # s-chunk for projections / attention query chunks
NSC = S // SC   # 4
NJB = S // P    # 16 key blocks
N_CORES = 8

f32 = mybir.dt.float32
f32r = mybir.dt.float32r
bf16 = mybir.dt.bfloat16

# matmul dtype: "f32r" (default), "f32", "bf16"
MM_DT_NAME = os.environ.get("KDT", "f32r")
_DT_MAP = {"f32r": f32r, "f32": f32, "bf16": bf16}
_NP_MAP = {"f32r": np.float32, "f32": np.float32}

NEG = -1.0e9


def _np_dt(name):
    if name == "bf16":
        import ml_dtypes

        return ml_dtypes.bfloat16
    return _NP_MAP[name]


def build_nc(dt_name=MM_DT_NAME):
    DT = _DT_MAP[dt_name]
    nc = bacc.Bacc("TRN2", target_bir_lowering=False, debug=False,
                   num_devices=N_CORES)

    xT = nc.declare_dram_parameter("xT", [D, S], DT, isOutput=False)
    # [wq(256) | wk(256) | wv(256) | wq_shift(256) | wk_shift(256)]
    w = nc.declare_dram_parameter("w", [D, 1280], DT, isOutput=False)
    wo = nc.declare_dram_parameter("wo", [NH * DH, D], DT, isOutput=False)
    cos2 = nc.declare_dram_parameter("cos2", [P, S], f32, isOutput=False)
    sin2 = nc.declare_dram_parameter("sin2", [P, S], f32, isOutput=False)
    bias = nc.declare_dram_parameter("bias", [P, 4 * SC], f32, isOutput=False)
    y = nc.declare_dram_parameter("y", [S, D], f32, isOutput=True)

    xT3 = xT.rearrange("(o p) s -> p o s", p=P)        # [128, 8, 2048]
    w3 = w.rearrange("(o p) n -> p o n", p=P)          # [128, 8, 1280]
    wo3 = wo.rearrange("(o p) n -> p o n", p=P)        # [128, 2, 1024]
    bias3 = bias.rearrange("p (k n) -> p k n", k=4)    # [128, 4, 512]
    y3 = y.rearrange("(o p) n -> p o n", p=P)          # [128, 16, 1024]

    Exp = mybir.ActivationFunctionType.Exp
    mult = mybir.AluOpType.mult

    with tile.TileContext(nc) as tc, ExitStack() as ctx:
        cpool = ctx.enter_context(tc.tile_pool(name="const", bufs=1))
        xpool = ctx.enter_context(tc.tile_pool(name="xin", bufs=2))
        qkpool = ctx.enter_context(tc.tile_pool(name="qk", bufs=1))
        vpool = ctx.enter_context(tc.tile_pool(name="vt", bufs=1))
        otpool = ctx.enter_context(tc.tile_pool(name="ot", bufs=1))
        rtmp = ctx.enter_context(tc.tile_pool(name="rtmp", bufs=3))
        ptpool = ctx.enter_context(tc.tile_pool(name="pt", bufs=4))
        npool = ctx.enter_context(tc.tile_pool(name="norm", bufs=2))
        opool = ctx.enter_context(tc.tile_pool(name="ostage", bufs=3))
        pp_proj = ctx.enter_context(
            tc.tile_pool(name="pproj", bufs=3, space="PSUM"))
        pp_attn = ctx.enter_context(
            tc.tile_pool(name="pattn", bufs=2, space="PSUM"))
        pp_avo = ctx.enter_context(
            tc.tile_pool(name="pavo", bufs=2, space="PSUM"))

        # ---- constants / weights ----
        w_sb = cpool.tile([P, KB, 1280], DT)
        nc.sync.dma_start(w_sb[:], w3)
        wo_sb = cpool.tile([P, 2, D], DT)
        nc.sync.dma_start(wo_sb[:], wo3)
        cos_sb = cpool.tile([P, S], f32)
        nc.sync.dma_start(cos_sb[:], cos2[:, :])
        sin_sb = cpool.tile([P, S], f32)
        nc.sync.dma_start(sin_sb[:], sin2[:, :])
        bias_sb = cpool.tile([P, 4, SC], f32)
        nc.sync.dma_start(bias_sb[:], bias3)

        # q/k channel-major with 2 heads stacked per 128 partitions:
        # qT[hq*64+d, pair, s] = rope(q)[head 2*pair+hq, d, s]
        qT = qkpool.tile([P, 2, S], DT, tag="qT")
        kT = qkpool.tile([P, 2, S], DT, tag="kT")
        # v natural layout per key block, with a ones column appended
        # (fuses the softmax denominator into the hq=0 AV matmul)
        v_sb = vpool.tile([P, NJB, NH, DH + 1], DT)
        one_f32 = cpool.tile([P, 1], f32)
        nc.vector.memset(one_f32[:], 1.0)
        nc.vector.tensor_copy(
            out=v_sb[:, :, :, DH],
            in_=one_f32[:, :, None].to_broadcast((P, NJB, NH)))
        # attention output, channel-major, 2 heads stacked (out-proj lhsT)
        oT = otpool.tile([P, 2, S], DT)

        # ---- phase 1: qkv projections + rope ----
        for sc in range(NSC):
            ssl = slice(sc * SC, (sc + 1) * SC)
            x_sb = xpool.tile([P, KB, SC], DT)
            nc.sync.dma_start(x_sb[:], xT3[:, :, ssl])
            # q/k: psum[j, s] = sum_d w[d, j] xT[d, s], channels 2 heads/blk
            for jb in range(4):
                dst, pair = (qT, jb) if jb < 2 else (kT, jb - 2)
                psA = pp_proj.tile([P, SC], f32, tag="proj")
                psB = pp_proj.tile([P, SC], f32, tag="proj")
                for kb in range(KB):
                    nc.tensor.matmul(
                        psA[:], lhsT=w_sb[:, kb, jb * P:(jb + 1) * P],
                        rhs=x_sb[:, kb], start=(kb == 0), stop=(kb == KB - 1))
                for kb in range(KB):
                    nc.tensor.matmul(
                        psB[:], lhsT=w_sb[:, kb, 768 + jb * P:768 + (jb + 1) * P],
                        rhs=x_sb[:, kb], start=(kb == 0), stop=(kb == KB - 1))
                # rope: dst = psA*cos + psB*sin'   (shift folded into psB's W)
                t2 = rtmp.tile([P, SC], f32, tag="t2")
                nc.vector.tensor_mul(out=t2[:], in0=psA[:], in1=cos_sb[:, ssl])
                t3 = rtmp.tile([P, SC], f32, tag="t3")
                nc.vector.tensor_mul(out=t3[:], in0=psB[:], in1=sin_sb[:, ssl])
                nc.vector.tensor_add(out=dst[:, pair, ssl], in0=t2[:], in1=t3[:])
            # v: psum[s, j] = sum_d xT[d, s] w_v[d, j]
            for m in range(4):
                ps = pp_proj.tile([P, 256], f32, tag="proj")
                for kb in range(KB):
                    nc.tensor.matmul(
                        ps[:], lhsT=x_sb[:, kb, m * P:(m + 1) * P],
                        rhs=w_sb[:, kb, 512:768],
                        start=(kb == 0), stop=(kb == KB - 1))
                sidx = sc * 4 + m
                for h in range(NH):
                    nc.vector.tensor_copy(
                        out=v_sb[:, sidx, h, 0:DH],
                        in_=ps[:, h * DH:(h + 1) * DH])

        # ---- phase 2: attention (scores transposed, key pos on psum rows) --
        for h in range(NH):
            hp, hq = h // 2, h % 2
            psl = slice(hq * DH, (hq + 1) * DH)   # partitions of this head
            for ic in range(NSC):
                isl = slice(ic * SC, (ic + 1) * SC)
                njb = (ic + 1) * 4
                ps_o = pp_avo.tile([P, SC], f32, tag="avo")
                for jb in range(njb):
                    ps_s = pp_attn.tile([P, SC], f32, tag="scores")
                    nc.tensor.matmul(
                        ps_s[:], lhsT=kT[psl, hp, jb * P:(jb + 1) * P],
                        rhs=qT[psl, hp, isl], start=True, stop=True)
                    k_diag = jb - ic * 4
                    if k_diag >= 0:
                        nc.vector.tensor_add(
                            out=ps_s[:], in0=ps_s[:], in1=bias_sb[:, k_diag])
                    pt = ptpool.tile([P, SC], DT)
                    nc.scalar.activation(pt[:], ps_s[:], Exp, scale=0.125)
                    # AV + denominator in one matmul: [v|ones] -> rows
                    # 0..63 = unnormalized out, row 64 = exp row-sums
                    nc.tensor.matmul(
                        ps_o[0:DH + 1],
                        lhsT=v_sb[:, jb, h, 0:DH + 1], rhs=pt[:],
                        start=(jb == 0), stop=(jb == njb - 1))
                # normalize by the exp row-sums (row 64 of ps_o); engines
                # allow 32-aligned partition-base shifts between in and out,
                # and partition_broadcast reads its input's partition 0 only
                ns = npool.tile([P, SC], f32, tag="ns")
                nc.scalar.activation(ns[0:1], ps_o[DH:DH + 1],
                                     mybir.ActivationFunctionType.Copy)
                rb = npool.tile([P, SC], f32, tag="rb")
                nc.gpsimd.partition_broadcast(rb[0:DH], ns[0:1])
                nc.vector.reciprocal(rb[0:DH], rb[0:DH])
                nc.vector.tensor_mul(
                    out=oT[psl, hp, isl], in0=ps_o[0:DH], in1=rb[0:DH])

        # ---- phase 3: out projection (partial; host sums over cores) ----
        for so in range(NJB):
            for oc in range(2):
                ps = pp_proj.tile([P, SC], f32, tag="proj")
                for hb in range(2):
                    nc.tensor.matmul(
                        ps[:], lhsT=oT[:, hb, so * P:(so + 1) * P],
                        rhs=wo_sb[:, hb, oc * SC:(oc + 1) * SC],
                        start=(hb == 0), stop=(hb == 1))
                ost = opool.tile([P, SC], f32)
                nc.any.tensor_copy(out=ost[:], in_=ps[:])
                nc.sync.dma_start(y3[:, so, oc * SC:(oc + 1) * SC], ost[:])

    nc.compile()
    return nc


def _host_inputs(x, w_qkv, w_out, freqs, dt_name=MM_DT_NAME):
    """Build the 8 per-core input maps."""
    npdt = _np_dt(dt_name)
    x = np.asarray(x, dtype=np.float32)
    w_qkv = np.asarray(w_qkv, dtype=np.float32)
    w_out = np.asarray(w_out, dtype=np.float32)
    freqs = np.asarray(freqs, dtype=np.float32)

    cosT = np.cos(freqs).T.astype(np.float32)          # [64, 2048]
    sinT = np.sin(freqs).T.astype(np.float32)
    sinTm = np.concatenate([-sinT[:32], sinT[32:]], axis=0)
    cos2 = np.ascontiguousarray(np.tile(cosT, (2, 1)))  # [128, 2048]
    sin2 = np.ascontiguousarray(np.tile(sinTm, (2, 1)))

    j = np.arange(P)[:, None]
    t = np.arange(P)[None, :]
    tri = np.where(j <= t, np.float32(0), np.float32(NEG))  # [128, 128]
    bias = np.zeros((P, 4, 4, P), dtype=np.float32)
    for k in range(4):
        bias[:, k, :k, :] = NEG
        bias[:, k, k, :] = tri
    bias = bias.reshape(P, 4 * SC)

    xTs = [np.ascontiguousarray(x[b].T).astype(npdt) for b in range(B)]

    def shift_cols(wm):
        # swap 32-halves within each head's 64 columns
        d, n = wm.shape
        return np.ascontiguousarray(
            wm.reshape(d, n // DH, 2, DH // 2)[:, :, ::-1, :].reshape(d, n))

    in_maps = []
    for c in range(N_CORES):
        b, hg = c // 4, c % 4
        cs = slice(hg * 256, (hg + 1) * 256)
        wq = w_qkv[:, 0 * D:1 * D][:, cs]
        wk = w_qkv[:, 1 * D:2 * D][:, cs]
        wv = w_qkv[:, 2 * D:3 * D][:, cs]
        w_s = np.concatenate(
            [wq, wk, wv, shift_cols(wq), shift_cols(wk)], axis=1).astype(npdt)
        wo_s = np.ascontiguousarray(w_out[hg * 256:(hg + 1) * 256, :]).astype(npdt)
        in_maps.append({
            "xT": xTs[b],
            "w": np.ascontiguousarray(w_s),
            "wo": wo_s,
            "cos2": cos2,
            "sin2": sin2,
            "bias": bias,
        })
    return in_maps


_CACHE = {}


def _get_runner():
    """Compile once per process; return a callable in_maps -> per-core y."""
    if "runner" in _CACHE:
        return _CACHE["runner"]

    import jax
    from jax.sharding import Mesh, PartitionSpec
    from jax.experimental.shard_map import shard_map
    from concourse import bass2jax

    bass2jax.install_neuronx_cc_hook()
    nc = build_nc()

    partition_name = (nc.partition_id_tensor.name
                      if nc.partition_id_tensor else None)
    in_names = []
    out_names = []
    out_avals = []
    zero_outs = []
    for alloc in nc.m.functions[0].allocations:
        if not isinstance(alloc, mybir.MemoryLocationSet):
            continue
        name = alloc.memorylocations[0].name
        if alloc.kind == "ExternalInput":
            if name != partition_name:
                in_names.append(name)
        elif alloc.kind == "ExternalOutput":
            shape = tuple(alloc.tensor_shape)
            dtype = mybir.dt.np(alloc.dtype)
            out_names.append(name)
            out_avals.append(jax.core.ShapedArray(shape, dtype))
            zero_outs.append(np.zeros(shape, dtype))
    n_params = len(in_names)
    n_outs = len(out_avals)
    all_names = in_names + out_names
    if partition_name is not None:
        all_names = all_names + [partition_name]

    def _body(*args):
        operands = list(args)
        if partition_name is not None:
            operands.append(bass2jax.partition_id_tensor())
        outs = bass2jax._bass_exec_p.bind(
            *operands,
            out_avals=tuple(out_avals),
            in_names=tuple(all_names),
            out_names=tuple(out_names),
            lowering_input_output_aliases=(),
            sim_require_finite=True,
            sim_require_nnan=True,
            nc=nc,
        )
        return tuple(outs)

    devices = jax.devices()[:N_CORES]
    assert len(devices) == N_CORES
    mesh = Mesh(np.asarray(devices), ("core",))
    in_specs = (PartitionSpec("core"),) * (n_params + n_outs)
    out_specs = (PartitionSpec("core"),) * n_outs
    donate = tuple(range(n_params, n_params + n_outs))
    sharded = jax.jit(
        shard_map(_body, mesh=mesh, in_specs=in_specs, out_specs=out_specs,
                  check_rep=False),
        donate_argnums=donate, keep_unused=True)

    def run(in_maps):
        per_core = [[np.asarray(m[name]) for name in in_names]
                    for m in in_maps]
        concat_in = [
            np.concatenate([per_core[c][i] for c in range(N_CORES)], axis=0)
            for i in range(n_params)
        ]
        concat_zeros = [
            np.zeros((N_CORES * z.shape[0], *z.shape[1:]), z.dtype)
            for z in zero_outs
        ]
        out_arrs = sharded(*concat_in, *concat_zeros)
        out_arrs = [np.asarray(a) for a in out_arrs]
        return [
            {name: out_arrs[i].reshape(N_CORES, *out_avals[i].shape)[c]
             for i, name in enumerate(out_names)}
            for c in range(N_CORES)
        ]

    _CACHE["runner"] = run
    return run


def kernel(x, w_qkv, w_out, freqs):
    run = _get_runner()
    in_maps = _host_inputs(x, w_qkv, w_out, freqs)
    results = run(in_maps)
    out = np.zeros((B, S, D), dtype=np.float32)
    for c in range(N_CORES):
        out[c // 4] += results[c]["y"]
    return out


if __name__ == "__main__":
    rng = np.random.default_rng(0)
    x = rng.standard_normal((B, S, D), dtype=np.float32)
    w_qkv = (rng.standard_normal((D, 3 * D), dtype=np.float32) * D ** -0.5)
    w_out = (rng.standard_normal((D, D), dtype=np.float32) * D ** -0.5)
    freqs = rng.standard_normal((S, DH), dtype=np.float32)
    y = kernel(x, w_qkv, w_out, freqs)
    print("out", y.shape, y.dtype, float(np.abs(y).max()))
